# revision 22
# baseline (speedup 1.0000x reference)
"""YOLOv3 head decode (DarkNet53.transform_grid_data) on 8 Trainium2 cores.

Input : features [32, 255, 76, 76] f32, anchor_size [6] f32
Output: [32, 17328, 85] f32, rows ordered (anchor, gy, gx), row layout
        [objness, box_x, box_y, box_w, box_h, conf*80].

Strategy: pure data-parallel over batch (4 batches/core). Per (batch,
anchor) plane the job is a [85, 5776] -> [5776, 85] f32 transpose with
pointwise transforms on 5 of 85 attr rows. Transpose runs on the PE
(identity matmul, SBUF->PSUM), DVE copies PSUM into an SBUF staging tile
whose partition p holds output rows [45p, 45p+45), so the store is one
fully contiguous DMA. The special attrs are fixed up post-transpose with
strided free-dim APs at full partition utilization:
  obj/x/y: sigmoid (ACT);  x/y: out = 8*sig + 8*grid  (fused DVE op)
  w/h:     8*anchor*exp(v) = exp(v + ln(8*anchor))    (ACT bias fold)
"""

import os
import sys

import numpy as np

try:
    import concourse.bass as bass
except ImportError:  # pragma: no cover
    sys.path.insert(0, "/opt/trn_rl_repo")
    import concourse.bass as bass

import concourse.bacc as bacc
import concourse.mybir as mybir
from concourse.bass_utils import run_bass_kernel_spmd
from concourse.tile import TileContext
from concourse.tile_rust import add_dep_helper

B = 32
A = 3
ATTR = 85
GH = GW = 76
NPIX = GH * GW            # 5776
STRIDE = 8                # 608 / 76
N_CORES = 8
B_LOC = B // N_CORES      # 4 batches per core
NPLANE = B_LOC * A        # 12 (batch, anchor) planes per core
K = 45                    # output rows per partition in the staging tile
NMAIN = 128 * K           # 5760 pixels via the main path
TAIL = NPIX - NMAIN       # 16 pixels via the tail path
QGRP = 5                  # transposes per PSUM bank (5*85*4B = 1700B < 2KB)

_f32 = mybir.dt.float32
_cache = {}


def _grid_xy8():
    """8*gx, 8*gy per pixel, in the staging layout [part, K, 2] + tail."""
    pix = np.arange(NPIX, dtype=np.int64)
    x8 = (STRIDE * (pix % GW)).astype(np.float32)
    y8 = (STRIDE * (pix // GW)).astype(np.float32)
    xy = np.stack([x8, y8], axis=-1)               # [5776, 2]
    main = xy[:NMAIN].reshape(128, K * 2)          # [128, 90]
    tail = xy[NMAIN:]                              # [16, 2]
    return np.ascontiguousarray(main), np.ascontiguousarray(tail)


def _build():
    # Bacc (not plain Bass): TRN2 instructions carry at most ONE sync wait;
    # Bacc.generate_event_semaphores splits the extras into event-semaphore
    # instructions at finalize time.
    nc = bacc.Bacc("TRN2", target_bir_lowering=False, debug=False)
    feat = nc.dram_tensor("feat", [NPLANE, ATTR, NPIX], _f32, kind="ExternalInput")
    biaswh = nc.dram_tensor("biaswh", [128, 2 * A], _f32, kind="ExternalInput")
    outp = nc.dram_tensor("out", [NPLANE, NPIX, ATTR], _f32, kind="ExternalOutput")

    xy_main_np, xy_tail_np = _grid_xy8()
    ident_h = nc.inline_tensor(np.eye(ATTR, dtype=np.float32), name="ident")
    xym_h = nc.inline_tensor(xy_main_np, name="xym")
    xyt_h = nc.inline_tensor(xy_tail_np, name="xyt")

    sig = mybir.ActivationFunctionType.Sigmoid
    exp = mybir.ActivationFunctionType.Exp
    mult = mybir.AluOpType.mult
    add = mybir.AluOpType.add

    with TileContext(nc) as tc:
        with (
            tc.tile_pool(name="consts", bufs=1) as cpool,
            tc.tile_pool(name="io", bufs=3) as iopool,
            tc.tile_pool(name="stg", bufs=2) as stpool,
            tc.tile_pool(name="ps", bufs=1, space="PSUM") as pspool,
            tc.tile_pool(name="pstail", bufs=1, space="PSUM") as ptpool,
            tc.tile_pool(name="pswarm", bufs=1, space="PSUM") as pwpool,
        ):
            id_t = cpool.tile([ATTR, ATTR], _f32)
            nc.sync.dma_start(out=id_t, in_=ident_h[:, :])
            bias_t = cpool.tile([128, 2 * A], _f32)
            nc.sync.dma_start(out=bias_t, in_=biaswh[:, :])
            xym_t = cpool.tile([128, K * 2], _f32)
            nc.sync.dma_start(out=xym_t, in_=xym_h[:, :])
            xyt_t = cpool.tile([TAIL, 2], _f32)
            nc.sync.dma_start(out=xyt_t, in_=xyt_h[:, :])
            xym3 = xym_t.rearrange("p (q c) -> p q c", c=2)

            # fp32 self-loading matmuls (no standalone LDWEIGHTS) can carry
            # only ONE sync wait in the S3_LW struct; walrus rejects more.
            # Real transposes would need two (PSUM WAW on PE completion +
            # WAR on the DVE copy that drained the slot). PE completions are
            # in-order, so dedicate throwaway 1-column transposes to single
            # waits: a per-group "carrier" whose PSUM-slot WAW self-wait
            # advances PE's observed completion clock past the previous
            # group, and a per-plane "absorber" that eats the input-DMA
            # wait. Real transposes then carry only the DVE WAR wait. The
            # whole PE stream is pinned in emission order (ordering-only
            # deps) so the add_semaphores clock walk sees this sequence.
            pe_chain = [None]

            def pe_t(out_ap, in_ap, ident, sync_on=None):
                inst = nc.tensor.transpose(out_ap, in_ap, ident)
                if pe_chain[0] is not None:
                    add_dep_helper(inst.ins, pe_chain[0].ins, sync=False,
                                   reason="pin PE order")
                if sync_on is not None:
                    add_dep_helper(inst.ins, sync_on.ins, sync=True,
                                   reason="carrier hosts PE completion wait")
                pe_chain[0] = inst
                return inst

            warm = pwpool.tile([1, 2 * ATTR], _f32, tag="warm")
            pe_t(warm[:, :ATTR], id_t[:, 0:1], id_t)
            last_mm = [None]

            for p in range(NPLANE):
                a = p % A
                in_t = iopool.tile([ATTR, NPIX], _f32, tag="in")
                nc.sync.dma_start(out=in_t, in_=feat[p])
                pe_t(warm[:, ATTR:], in_t[:, 0:1], id_t)
                # [85, 128, K]: dim1 = staging partition, dim2 = row in part
                in_v = in_t[:, :NMAIN].rearrange("k (n q) -> k n q", q=K)

                st = stpool.tile([128, K * ATTR], _f32, tag="st")
                for g in range(K // QGRP):
                    pe_t(warm[:, :ATTR], id_t[:, 0:1], id_t, sync_on=last_mm[0])
                    ps_t = pspool.tile([128, QGRP * ATTR], _f32, tag="ps")
                    for i in range(QGRP):
                        q = g * QGRP + i
                        mm = pe_t(
                            ps_t[:, i * ATTR : (i + 1) * ATTR], in_v[:, :, q], id_t
                        )
                    last_mm[0] = mm
                    nc.vector.tensor_copy(
                        st[:, g * QGRP * ATTR : (g + 1) * QGRP * ATTR], ps_t
                    )

                st3 = st.rearrange("n (q t) -> n q t", t=ATTR)
                nc.scalar.activation(st3[:, :, 0:3], st3[:, :, 0:3], sig)
                nc.scalar.activation(
                    st3[:, :, 3:4], st3[:, :, 3:4], exp, bias=bias_t[:, 2 * a : 2 * a + 1]
                )
                nc.scalar.activation(
                    st3[:, :, 4:5], st3[:, :, 4:5], exp,
                    bias=bias_t[:, 2 * a + 1 : 2 * a + 2],
                )
                nc.vector.scalar_tensor_tensor(
                    st3[:, :, 1:3], st3[:, :, 1:3], 8.0, xym3, op0=mult, op1=add
                )
                nc.sync.dma_start(
                    out=outp[p, :NMAIN, :].rearrange("(n q) t -> n q t", q=K),
                    in_=st3,
                )

                # 16-pixel tail
                pt_t = ptpool.tile([TAIL, ATTR], _f32, tag="pt")
                pe_t(pt_t, in_t[:, NMAIN:], id_t)
                tl = stpool.tile([TAIL, ATTR], _f32, tag="tl")
                nc.vector.tensor_copy(tl, pt_t)
                nc.scalar.activation(tl[:, 0:3], tl[:, 0:3], sig)
                nc.scalar.activation(
                    tl[:, 3:4], tl[:, 3:4], exp, bias=bias_t[:TAIL, 2 * a : 2 * a + 1]
                )
                nc.scalar.activation(
                    tl[:, 4:5], tl[:, 4:5], exp,
                    bias=bias_t[:TAIL, 2 * a + 1 : 2 * a + 2],
                )
                nc.vector.scalar_tensor_tensor(
                    tl[:, 1:3], tl[:, 1:3], 8.0, xyt_t, op0=mult, op1=add
                )
                nc.sync.dma_start(out=outp[p, NMAIN:, :], in_=tl)
    nc.finalize()
    return nc


def _get_nc():
    if "nc" not in _cache:
        _cache["nc"] = _build()
    return _cache["nc"]


def run(features, anchor_size, trace=False, **spmd_kwargs):
    features = np.ascontiguousarray(np.asarray(features, dtype=np.float32))
    anchor_size = np.asarray(anchor_size, dtype=np.float32)
    nc = _get_nc()

    # bias for the exp fold: w/h attr gets exp(v + ln(8*anchor))
    bias = np.log(8.0 * anchor_size.astype(np.float64)).astype(np.float32)
    biaswh = np.broadcast_to(bias, (128, 2 * A)).copy()

    in_maps = []
    for c in range(N_CORES):
        in_maps.append(
            {
                "feat": features[c * B_LOC : (c + 1) * B_LOC].reshape(
                    NPLANE, ATTR, NPIX
                ),
                "biaswh": biaswh,
            }
        )
    res = run_bass_kernel_spmd(
        nc, in_maps, list(range(N_CORES)), trace=trace, **spmd_kwargs
    )
    out = np.concatenate(
        [r["out"].reshape(B_LOC, A * NPIX, ATTR) for r in res.results], axis=0
    )
    return out, res


def kernel(features, anchor_size):
    out, _ = run(features, anchor_size)
    return out


def _prep_inputs(features, anchor_size):
    features = np.ascontiguousarray(np.asarray(features, dtype=np.float32))
    anchor_size = np.asarray(anchor_size, dtype=np.float32)
    bias = np.log(8.0 * anchor_size.astype(np.float64)).astype(np.float32)
    biaswh = np.broadcast_to(bias, (128, 2 * A)).copy()
    feats = [
        features[c * B_LOC : (c + 1) * B_LOC].reshape(NPLANE, ATTR, NPIX)
        for c in range(N_CORES)
    ]
    return feats, biaswh


def bench(features, anchor_size, iters=16):
    """Device-side exec time via K chained effectful executions in one jit.

    Returns (exec_ns_per_iter, out) where out is from the single-exec run.
    """
    import jax
    from jax.sharding import Mesh, PartitionSpec
    from jax.experimental.shard_map import shard_map
    import time

    from concourse import bass2jax as b2j

    nc = _get_nc()
    b2j.install_neuronx_cc_hook()

    part_name = nc.partition_id_tensor.name if nc.partition_id_tensor else None
    in_names, out_names, out_avals, zero_outs = [], [], [], []
    for alloc in nc.m.functions[0].allocations:
        if not isinstance(alloc, mybir.MemoryLocationSet):
            continue
        name = alloc.memorylocations[0].name
        if alloc.kind == "ExternalInput":
            if name != part_name:
                in_names.append(name)
        elif alloc.kind == "ExternalOutput":
            out_names.append(name)
            shape = tuple(alloc.tensor_shape)
            dtype = mybir.dt.np(alloc.dtype)
            out_avals.append(jax.core.ShapedArray(shape, dtype))
            zero_outs.append(np.zeros(shape, dtype))
    n_params = len(in_names)
    all_names = in_names + out_names

    bind_names = all_names + ([part_name] if part_name else [])

    def make_body(n_exec):
        def _body(*args):
            operands = list(args)
            if part_name:
                operands.append(b2j.partition_id_tensor())
            outs = None
            for _ in range(n_exec):
                outs = b2j._bass_exec_p.bind(
                    *operands,
                    out_avals=tuple(out_avals),
                    in_names=tuple(bind_names),
                    out_names=tuple(out_names),
                    lowering_input_output_aliases=(),
                    sim_require_finite=True,
                    sim_require_nnan=True,
                    nc=nc,
                )
            return tuple(outs)

        return _body

    devices = jax.devices()[:N_CORES]
    mesh = Mesh(np.asarray(devices), ("core",))
    nin = n_params + len(zero_outs)

    feats, biaswh = _prep_inputs(features, anchor_size)
    per_core = {"feat": feats, "biaswh": [biaswh] * N_CORES}
    concat_in = [
        np.concatenate(per_core[name], axis=0) for name in in_names
    ]
    concat_zero = [
        np.zeros((N_CORES * z.shape[0], *z.shape[1:]), z.dtype) for z in zero_outs
    ]

    fns = {}
    for k in (1, iters):
        fns[k] = jax.jit(
            shard_map(
                make_body(k),
                mesh=mesh,
                in_specs=(PartitionSpec("core"),) * nin,
                out_specs=(PartitionSpec("core"),) * len(out_names),
                check_rep=False,
            ),
            keep_unused=True,
        )

    args = concat_in + concat_zero
    # warm both executables
    out1 = fns[1](*args)
    jax.block_until_ready(out1)
    outk = fns[iters](*args)
    jax.block_until_ready(outk)

    def t(f):
        best = float("inf")
        for _ in range(3):
            t0 = time.perf_counter()
            jax.block_until_ready(f(*args))
            best = min(best, time.perf_counter() - t0)
        return best

    t1 = t(fns[1])
    tk = t(fns[iters])
    exec_ns = (tk - t1) / (iters - 1) * 1e9
    out = np.concatenate(
        [
            np.asarray(out1[0]).reshape(N_CORES, *out_avals[0].shape)[c].reshape(
                B_LOC, A * NPIX, ATTR
            )
            for c in range(N_CORES)
        ],
        axis=0,
    )
    return exec_ns, out, (t1, tk)
